# revision 6
# baseline (speedup 1.0000x reference)
import sys
import numpy as np

sys.path.insert(0, '/opt/trn_rl_repo')

import ml_dtypes
import concourse.bacc as bacc
import concourse.mybir as mybir
from concourse.bass_utils import run_bass_kernel_spmd
from concourse.tile import TileContext
from contextlib import ExitStack

f32 = mybir.dt.float32
f32r = mybir.dt.float32r
bf16 = mybir.dt.bfloat16
AF = mybir.ActivationFunctionType
ALU = mybir.AluOpType

D_MODEL = 1024
N_HEAD = 16
D_HEAD = 64
B = 4
T = 2048
N_CORES = 8
NP = 4            # head pairs per core
NKC = 8           # 128-row chunks over model dim (contraction)
NKB = 32          # key blocks of 64

_cache = {}


def _build():
    nc = bacc.Bacc()
    xT = nc.declare_dram_parameter("xT", [D_MODEL, T], bf16, isOutput=False)
    wqT = nc.declare_dram_parameter("wqT", [D_MODEL, 512], bf16, isOutput=False)
    wkT = nc.declare_dram_parameter("wkT", [D_MODEL, 512], bf16, isOutput=False)
    wvT = nc.declare_dram_parameter("wvT", [D_MODEL, 512], bf16, isOutput=False)
    wpT = nc.declare_dram_parameter("wpT", [512, D_MODEL], bf16, isOutput=False)
    dmask = nc.declare_dram_parameter("dmask", [128, 64], bf16, isOutput=False)
    onesbd = nc.declare_dram_parameter("onesbd", [128, 2], bf16, isOutput=False)
    ones2t = nc.declare_dram_parameter("ones2t", [2, 128], f32r, isOutput=False)
    outp = nc.declare_dram_parameter("out", [T, D_MODEL], f32, isOutput=True)

    with TileContext(nc) as tc, ExitStack() as X:
        pers = X.enter_context(tc.tile_pool(name="pers", bufs=1))
        xp = X.enter_context(tc.tile_pool(name="x", bufs=1))
        ptp = X.enter_context(tc.tile_pool(name="pt", bufs=4))
        psp = X.enter_context(tc.tile_pool(name="ptsum", bufs=2))
        rcp = X.enter_context(tc.tile_pool(name="rc", bufs=2))
        obp = X.enter_context(tc.tile_pool(name="ob", bufs=2))
        psaP = X.enter_context(tc.tile_pool(name="psa", bufs=2, space="PSUM"))
        psyP = X.enter_context(tc.tile_pool(name="psy", bufs=2, space="PSUM"))
        scrP = X.enter_context(tc.tile_pool(name="scr", bufs=2, space="PSUM"))

        qp = [pers.tile([128, T], bf16, name=f"qp{m}") for m in range(NP)]
        kbd = [pers.tile([128, NKB * 128], bf16, name=f"kbd{m}") for m in range(NP)]
        vbd = [pers.tile([128, NKB * 128], bf16, name=f"vbd{m}") for m in range(NP)]
        ysb = [pers.tile([128, T], bf16, name=f"ysb{m}") for m in range(NP)]
        vsb = pers.tile([128, 16 * 512], bf16, name="vsb")
        wqt = [pers.tile([128, 512], bf16, name=f"wqt{k}") for k in range(NKC)]
        wkt = [pers.tile([128, 512], bf16, name=f"wkt{k}") for k in range(NKC)]
        wvt = [pers.tile([128, 512], bf16, name=f"wvt{k}") for k in range(NKC)]
        wpt = [pers.tile([128, D_MODEL], bf16, name=f"wpt{k}") for k in range(4)]
        msk = pers.tile([128, 64], bf16, name="msk")
        ob1 = pers.tile([128, 2], bf16, name="ob1")
        o2t = pers.tile([2, 128], f32r, name="o2t")

        # kbd block kb: [128, (2,64)] = block-diag(kA keys, kB keys) over d-partitions
        kbd3 = [kbd[m][:].rearrange("p (kb two s) -> p kb two s", two=2, s=64)
                for m in range(NP)]
        # vbd block kb=(tb,par): [128,128] = block-diag(vA, vB) over key-partitions
        vbd4 = [vbd[m][:].rearrange("p (tb par f) -> p tb par f", par=2, f=128)
                for m in range(NP)]
        vsb2 = vsb[:].rearrange("p (tb c) -> p tb c", tb=16)
        vsb3 = vsb[:].rearrange("p (tb m h d) -> p tb m h d", tb=16, m=4, h=2, d=64)

        # zero the off-diagonal quadrants once, before S1 evictions land
        for m in range(NP):
            nc.gpsimd.memset(kbd3[m][0:64, :, 1, :], 0.0)
            nc.gpsimd.memset(kbd3[m][64:128, :, 0, :], 0.0)
            nc.vector.memset(vbd4[m][0:64, :, :, 64:128], 0.0)
            nc.vector.memset(vbd4[m][64:128, :, :, 0:64], 0.0)

        nc.sync.dma_start(out=msk[:], in_=dmask[:, :])
        nc.sync.dma_start(out=ob1[:], in_=onesbd[:, :])
        nc.sync.dma_start(out=o2t[:], in_=ones2t[:, :])
        for k in range(NKC):
            nc.scalar.dma_start(out=wqt[k][:], in_=wqT[k * 128:(k + 1) * 128, :])
            nc.scalar.dma_start(out=wkt[k][:], in_=wkT[k * 128:(k + 1) * 128, :])
            nc.scalar.dma_start(out=wvt[k][:], in_=wvT[k * 128:(k + 1) * 128, :])
        for k in range(4):
            nc.scalar.dma_start(out=wpt[k][:], in_=wpT[k * 128:(k + 1) * 128, :])

        def load_x(th):
            ts = []
            for k in range(NKC):
                t_ = xp.tile([128, 1024], bf16, tag=f"x{k}", name=f"x{k}_{th}")
                nc.sync.dma_start(out=t_[:],
                                  in_=xT[k * 128:(k + 1) * 128,
                                         th * 1024:(th + 1) * 1024])
                ts.append(t_)
            return ts

        def s1_qk(th, m, xt):
            hb = th * 1024
            for which in ("q", "k"):
                wt = wqt if which == "q" else wkt
                ps = psaP.tile([128, 1024], f32, tag="psa", name=f"ps{which}{m}_{th}")
                for kc in range(NKC):
                    for j in range(2):
                        nc.tensor.matmul(
                            ps[:, j * 512:(j + 1) * 512],
                            wt[kc][:, m * 128:(m + 1) * 128],
                            xt[kc][:, j * 512:(j + 1) * 512],
                            start=(kc == 0), stop=(kc == NKC - 1))
                if which == "q":
                    nc.vector.tensor_copy(qp[m][:, hb:hb + 1024], ps[:])
                else:
                    ps3 = ps[:].rearrange("p (kb s) -> p kb s", s=64)
                    nc.vector.tensor_copy(
                        kbd3[m][0:64, th * 16:(th + 1) * 16, 0, :], ps3[0:64])
                    nc.vector.tensor_copy(
                        kbd3[m][64:128, th * 16:(th + 1) * 16, 1, :], ps3[64:128])

        def s1_v(th, tl, xt):
            tb = th * 8 + tl
            ps = scrP.tile([128, 512], f32, tag="scr", name=f"psv{tb}")
            for kc in range(NKC):
                nc.tensor.matmul(ps[:], xt[kc][:, tl * 128:(tl + 1) * 128],
                                 wvt[kc][:], start=(kc == 0), stop=(kc == NKC - 1))
            nc.vector.tensor_copy(vsb2[:, tb, :], ps[:])

        def s1_vshuffle(th, m):
            tbs = slice(th * 8, (th + 1) * 8)
            nc.gpsimd.tensor_copy(vbd4[m][0:64, tbs, 0, 0:64],
                                  vsb3[0:64, tbs, m, 0, :])
            nc.gpsimd.tensor_copy(vbd4[m][64:128, tbs, 1, 64:128],
                                  vsb3[64:128, tbs, m, 1, :])
            nc.sync.dma_start(out=vbd4[m][64:128, tbs, 0, 64:128],
                              in_=vsb3[0:64, tbs, m, 1, :])
            nc.sync.dma_start(out=vbd4[m][0:64, tbs, 1, 0:64],
                              in_=vsb3[64:128, tbs, m, 0, :])

        pending = []

        def flush_pending():
            for f in pending:
                f()
            pending.clear()

        def s2_chain(m, jp):
            for u in range(2):
                flush_pending()
                tstart = jp * 1024 + u * 512
                kbmax = 16 * jp + 8 * u + 8
                psy_u = psyP.tile([128, 512], f32, tag="psy", name=f"psy{m}_{jp}_{u}")
                ptsum = psp.tile([128, 512], bf16, tag="ptsum", name=f"pts{m}_{jp}_{u}")
                for pi in range(0, kbmax, 2):
                    pair = [kb for kb in (pi, pi + 1) if kb < kbmax]
                    psa = psaP.tile([128, 1024], f32, tag="psa", name=f"psa{m}_{jp}_{u}_{pi}")
                    pt = ptp.tile([128, 1024], bf16, tag="pt", name=f"pt{m}_{jp}_{u}_{pi}")
                    sus = []
                    for h, kb in enumerate(pair):
                        su = max(0, 64 * kb - tstart)
                        sus.append(su)
                        nc.tensor.matmul(
                            psa[:, h * 512 + su:(h + 1) * 512],
                            kbd3[m][:, kb, :, :],
                            qp[m][:, tstart + su:tstart + 512],
                            start=True, stop=True)
                    if len(pair) == 2 and sus[0] == sus[1]:
                        p3i = psa[:].rearrange("p (g c) -> p g c", g=2)
                        p3o = pt[:].rearrange("p (g c) -> p g c", g=2)
                        nc.scalar.activation(p3o[:, :, sus[0]:512],
                                             p3i[:, :, sus[0]:512], AF.Exp)
                    else:
                        for h, kb in enumerate(pair):
                            su = sus[h]
                            nc.scalar.activation(
                                pt[:, h * 512 + su:(h + 1) * 512],
                                psa[:, h * 512 + su:(h + 1) * 512], AF.Exp)
                    for h, kb in enumerate(pair):
                        su = sus[h]
                        if 64 * kb >= tstart:
                            dc = h * 512 + 64 * kb - tstart
                            nc.gpsimd.tensor_tensor(pt[:, dc:dc + 64],
                                                    pt[:, dc:dc + 64],
                                                    msk[:], ALU.mult)
                        if kb == 0:
                            nc.gpsimd.tensor_copy(ptsum[:, 0:512], pt[:, 0:512])
                        else:
                            nc.gpsimd.tensor_tensor(
                                ptsum[:, su:512], ptsum[:, su:512],
                                pt[:, h * 512 + su:(h + 1) * 512], ALU.add)
                        nc.tensor.matmul(
                            psy_u[:, su:512],
                            vbd4[m][:, kb // 2, kb % 2, :],
                            pt[:, h * 512 + su:(h + 1) * 512],
                            start=(kb == 0), stop=(kb == kbmax - 1),
                            skip_group_check=True)

                def tail(m=m, tstart=tstart, psy_u=psy_u, ptsum=ptsum):
                    psd = scrP.tile([128, 512], f32, tag="scr", name="psd")
                    nc.tensor.matmul(psd[0:2, :], ob1[:], ptsum[:, 0:512],
                                     start=True, stop=True)
                    rc = rcp.tile([2, 512], f32r, tag="rc", name="rct")
                    with nc.allow_low_precision(reason="f32r is full-precision bits"):
                        nc.vector.reciprocal(rc[:], psd[0:2, :])
                    bc = scrP.tile([128, 512], f32, tag="scr", name="bct")
                    nc.tensor.matmul(bc[:], o2t[:], rc[:], start=True, stop=True)
                    bcs = rcp.tile([128, 512], f32, tag="bcs", name="bcst")
                    nc.vector.tensor_copy(bcs[:], bc[:])
                    nc.vector.tensor_tensor(ysb[m][:, tstart:tstart + 512],
                                            psy_u[:], bcs[:], ALU.mult)
                pending.append(tail)

        def s4(jp):
            flush_pending()
            for tb in range(8):
                t0 = jp * 8 + tb
                ps = psaP.tile([128, 1024], f32, tag="psa", name=f"ps4_{t0}")
                for oc in range(2):
                    for k in range(4):
                        nc.tensor.matmul(
                            ps[:, oc * 512:(oc + 1) * 512],
                            ysb[k][:, t0 * 128:(t0 + 1) * 128],
                            wpt[k][:, oc * 512:(oc + 1) * 512],
                            start=(k == 0), stop=(k == 3))
                o_ = obp.tile([128, 1024], f32, tag="ob", name=f"ob{t0}")
                nc.vector.tensor_copy(o_[:], ps[:])
                nc.scalar.dma_start(out=outp[t0 * 128:(t0 + 1) * 128, :], in_=o_[:])

        # ---- master schedule ----
        xt0 = load_x(0)
        for m in range(NP):
            s1_qk(0, m, xt0)
        for tl in range(8):
            s1_v(0, tl, xt0)
        for m in range(NP):
            s1_vshuffle(0, m)
        xt1 = load_x(1)
        # jp0 attention interleaved with th1 projections (pure-PE filler)
        for m in range(NP):
            s2_chain(m, 0)
            s1_qk(1, m, xt1)
            s1_v(1, 2 * m, xt1)
            s1_v(1, 2 * m + 1, xt1)
        for m in range(NP):
            s1_vshuffle(1, m)
        s4(0)
        for m in range(NP):
            s2_chain(m, 1)
        s4(1)
        flush_pending()

    nc.compile()
    return nc


def _prep_core_inputs(x, w_qkv, w_proj, c):
    b, g = c // 2, c % 2
    bf = ml_dtypes.bfloat16
    scale = np.float32(D_HEAD ** -0.5)
    wq = w_qkv[512 * g:512 * (g + 1)] * scale
    wk = w_qkv[D_MODEL + 512 * g:D_MODEL + 512 * (g + 1)]
    wv = w_qkv[2 * D_MODEL + 512 * g:2 * D_MODEL + 512 * (g + 1)]
    r = np.arange(128) % 64
    j = np.arange(64)
    dmask = (r[:, None] <= j[None, :]).astype(np.float32)
    onesbd = np.zeros((128, 2), np.float32)
    onesbd[0:64, 0] = 1.0
    onesbd[64:128, 1] = 1.0
    ones2t = np.zeros((2, 128), np.float32)
    ones2t[0, 0:64] = 1.0
    ones2t[1, 64:128] = 1.0
    return {
        "xT": np.ascontiguousarray(x[b].T).astype(bf),
        "wqT": np.ascontiguousarray(wq.T).astype(bf),
        "wkT": np.ascontiguousarray(wk.T).astype(bf),
        "wvT": np.ascontiguousarray(wv.T).astype(bf),
        "wpT": np.ascontiguousarray(w_proj[:, 512 * g:512 * (g + 1)].T).astype(bf),
        "dmask": dmask.astype(bf),
        "onesbd": onesbd.astype(bf),
        "ones2t": ones2t,
    }


def kernel(x, w_qkv, w_proj):
    x = np.asarray(x)
    w_qkv = np.asarray(w_qkv)
    w_proj = np.asarray(w_proj)
    if "nc" not in _cache:
        _cache["nc"] = _build()
    nc = _cache["nc"]
    in_maps = [_prep_core_inputs(x, w_qkv, w_proj, c) for c in range(N_CORES)]
    res = run_bass_kernel_spmd(nc, in_maps, core_ids=list(range(N_CORES)))
    outs = [res.results[c]["out"] for c in range(N_CORES)]
    return np.stack([outs[2 * b] + outs[2 * b + 1] for b in range(B)], 0)


# revision 8
# speedup vs baseline: 1.4127x; 1.4127x over previous
import sys
import numpy as np

sys.path.insert(0, '/opt/trn_rl_repo')

import ml_dtypes
import concourse.bacc as bacc
import concourse.mybir as mybir
from concourse.bass_utils import run_bass_kernel_spmd
from concourse.tile import TileContext
from contextlib import ExitStack

f32 = mybir.dt.float32
f32r = mybir.dt.float32r
bf16 = mybir.dt.bfloat16
AF = mybir.ActivationFunctionType
ALU = mybir.AluOpType

D_MODEL = 1024
N_HEAD = 16
D_HEAD = 64
B = 4
T = 2048
N_CORES = 8
NP = 4            # head pairs per core
NKC = 8           # 128-row chunks over model dim (contraction)
NKB = 32          # key blocks of 64

_cache = {}


def _build():
    nc = bacc.Bacc()
    xT = nc.declare_dram_parameter("xT", [D_MODEL, T], bf16, isOutput=False)
    wqT = nc.declare_dram_parameter("wqT", [D_MODEL, 512], bf16, isOutput=False)
    wkT = nc.declare_dram_parameter("wkT", [D_MODEL, 512], bf16, isOutput=False)
    wvT = nc.declare_dram_parameter("wvT", [D_MODEL, 512], bf16, isOutput=False)
    wpT = nc.declare_dram_parameter("wpT", [512, D_MODEL], bf16, isOutput=False)
    dmask = nc.declare_dram_parameter("dmask", [128, 64], bf16, isOutput=False)
    onesbd = nc.declare_dram_parameter("onesbd", [128, 2], bf16, isOutput=False)
    ones2t = nc.declare_dram_parameter("ones2t", [2, 128], f32r, isOutput=False)
    outp = nc.declare_dram_parameter("out", [T, D_MODEL], f32, isOutput=True)

    with TileContext(nc) as tc, ExitStack() as X:
        pers = X.enter_context(tc.tile_pool(name="pers", bufs=1))
        xp = X.enter_context(tc.tile_pool(name="x", bufs=1))
        ptp = X.enter_context(tc.tile_pool(name="pt", bufs=4))
        psp = X.enter_context(tc.tile_pool(name="ptsum", bufs=2))
        rcp = X.enter_context(tc.tile_pool(name="rc", bufs=2))
        obp = X.enter_context(tc.tile_pool(name="ob", bufs=2))
        psaP = X.enter_context(tc.tile_pool(name="psa", bufs=2, space="PSUM"))
        psyP = X.enter_context(tc.tile_pool(name="psy", bufs=2, space="PSUM"))
        scrP = X.enter_context(tc.tile_pool(name="scr", bufs=2, space="PSUM"))

        qp = [pers.tile([128, T], bf16, name=f"qp{m}") for m in range(NP)]
        kbd = [pers.tile([128, NKB * 128], bf16, name=f"kbd{m}") for m in range(NP)]
        vbd = [pers.tile([128, NKB * 128], bf16, name=f"vbd{m}") for m in range(NP)]
        ysb = [pers.tile([128, T], bf16, name=f"ysb{m}") for m in range(NP)]
        vsb = pers.tile([128, 16 * 512], bf16, name="vsb")
        wqt = [pers.tile([128, 512], bf16, name=f"wqt{k}") for k in range(NKC)]
        wkt = [pers.tile([128, 512], bf16, name=f"wkt{k}") for k in range(NKC)]
        wvt = [pers.tile([128, 512], bf16, name=f"wvt{k}") for k in range(NKC)]
        wpt = [pers.tile([128, D_MODEL], bf16, name=f"wpt{k}") for k in range(4)]
        msk = pers.tile([128, 64], bf16, name="msk")
        ob1 = pers.tile([128, 2], bf16, name="ob1")
        o2t = pers.tile([2, 128], f32r, name="o2t")

        # kbd block kb: [128, (2,64)] = block-diag(kA keys, kB keys) over d-partitions
        kbd3 = [kbd[m][:].rearrange("p (kb two s) -> p kb two s", two=2, s=64)
                for m in range(NP)]
        # vbd block kb=(tb,par): [128,128] = block-diag(vA, vB) over key-partitions
        vbd4 = [vbd[m][:].rearrange("p (tb par f) -> p tb par f", par=2, f=128)
                for m in range(NP)]
        vsb2 = vsb[:].rearrange("p (tb c) -> p tb c", tb=16)
        vsb3 = vsb[:].rearrange("p (tb m h d) -> p tb m h d", tb=16, m=4, h=2, d=64)

        # zero the off-diagonal quadrants once, before S1 evictions land
        for m in range(NP):
            nc.gpsimd.memset(kbd3[m][0:64, :, 1, :], 0.0)
            nc.gpsimd.memset(kbd3[m][64:128, :, 0, :], 0.0)
            nc.gpsimd.memset(vbd4[m][0:64, :, :, 64:128], 0.0)
            nc.gpsimd.memset(vbd4[m][64:128, :, :, 0:64], 0.0)

        nc.sync.dma_start(out=msk[:], in_=dmask[:, :])
        nc.sync.dma_start(out=ob1[:], in_=onesbd[:, :])
        nc.sync.dma_start(out=o2t[:], in_=ones2t[:, :])
        for k in range(NKC):
            nc.scalar.dma_start(out=wqt[k][:], in_=wqT[k * 128:(k + 1) * 128, :])
            nc.scalar.dma_start(out=wkt[k][:], in_=wkT[k * 128:(k + 1) * 128, :])
            nc.scalar.dma_start(out=wvt[k][:], in_=wvT[k * 128:(k + 1) * 128, :])
        for k in range(4):
            nc.scalar.dma_start(out=wpt[k][:], in_=wpT[k * 128:(k + 1) * 128, :])

        def load_x(th):
            ts = []
            for k in range(NKC):
                t_ = xp.tile([128, 1024], bf16, tag=f"x{k}", name=f"x{k}_{th}")
                nc.sync.dma_start(out=t_[:],
                                  in_=xT[k * 128:(k + 1) * 128,
                                         th * 1024:(th + 1) * 1024])
                ts.append(t_)
            return ts

        def s1_qk(th, m, xt):
            hb = th * 1024
            for which in ("q", "k"):
                wt = wqt if which == "q" else wkt
                ps = psaP.tile([128, 1024], f32, tag="psa", name=f"ps{which}{m}_{th}")
                for kc in range(NKC):
                    for j in range(2):
                        nc.tensor.matmul(
                            ps[:, j * 512:(j + 1) * 512],
                            wt[kc][:, m * 128:(m + 1) * 128],
                            xt[kc][:, j * 512:(j + 1) * 512],
                            start=(kc == 0), stop=(kc == NKC - 1))
                if which == "q":
                    nc.scalar.activation(qp[m][:, hb:hb + 1024], ps[:], AF.Copy)
                else:
                    ps3 = ps[:].rearrange("p (kb s) -> p kb s", s=64)
                    nc.scalar.activation(
                        kbd3[m][0:64, th * 16:(th + 1) * 16, 0, :], ps3[0:64], AF.Copy)
                    nc.scalar.activation(
                        kbd3[m][64:128, th * 16:(th + 1) * 16, 1, :], ps3[64:128], AF.Copy)

        def s1_v(th, tl, xt):
            tb = th * 8 + tl
            ps = scrP.tile([128, 512], f32, tag="scr", name=f"psv{tb}")
            for kc in range(NKC):
                nc.tensor.matmul(ps[:], xt[kc][:, tl * 128:(tl + 1) * 128],
                                 wvt[kc][:], start=(kc == 0), stop=(kc == NKC - 1))
            nc.scalar.activation(vsb2[:, tb, :], ps[:], AF.Copy)

        def s1_vshuffle(th, m):
            tbs = slice(th * 8, (th + 1) * 8)
            nc.sync.dma_start(out=vbd4[m][0:64, tbs, 0, 0:64],
                              in_=vsb3[0:64, tbs, m, 0, :])
            nc.sync.dma_start(out=vbd4[m][64:128, tbs, 1, 64:128],
                              in_=vsb3[64:128, tbs, m, 1, :])
            nc.sync.dma_start(out=vbd4[m][64:128, tbs, 0, 64:128],
                              in_=vsb3[0:64, tbs, m, 1, :])
            nc.sync.dma_start(out=vbd4[m][0:64, tbs, 1, 0:64],
                              in_=vsb3[64:128, tbs, m, 0, :])

        pending = []

        def flush_pending():
            for f in pending:
                f()
            pending.clear()

        def s2_chain(m, jp):
            for u in range(2):
                flush_pending()
                tstart = jp * 1024 + u * 512
                kbmax = 16 * jp + 8 * u + 8
                psy_u = psyP.tile([128, 512], f32, tag="psy", name=f"psy{m}_{jp}_{u}")
                ptsum = psp.tile([128, 512], bf16, tag="ptsum", name=f"pts{m}_{jp}_{u}")
                for pi in range(0, kbmax, 2):
                    pair = [kb for kb in (pi, pi + 1) if kb < kbmax]
                    psa = psaP.tile([128, 1024], f32, tag="psa", name=f"psa{m}_{jp}_{u}_{pi}")
                    pt = ptp.tile([128, 1024], bf16, tag="pt", name=f"pt{m}_{jp}_{u}_{pi}")
                    sus = []
                    for h, kb in enumerate(pair):
                        su = max(0, 64 * kb - tstart)
                        sus.append(su)
                        nc.tensor.matmul(
                            psa[:, h * 512 + su:(h + 1) * 512],
                            kbd3[m][:, kb, :, :],
                            qp[m][:, tstart + su:tstart + 512],
                            start=True, stop=True)
                    if len(pair) == 2 and sus[0] == sus[1]:
                        p3i = psa[:].rearrange("p (g c) -> p g c", g=2)
                        p3o = pt[:].rearrange("p (g c) -> p g c", g=2)
                        nc.scalar.activation(p3o[:, :, sus[0]:512],
                                             p3i[:, :, sus[0]:512], AF.Exp)
                    else:
                        for h, kb in enumerate(pair):
                            su = sus[h]
                            nc.scalar.activation(
                                pt[:, h * 512 + su:(h + 1) * 512],
                                psa[:, h * 512 + su:(h + 1) * 512], AF.Exp)
                    for h, kb in enumerate(pair):
                        su = sus[h]
                        if 64 * kb >= tstart:
                            dc = h * 512 + 64 * kb - tstart
                            nc.gpsimd.tensor_tensor(pt[:, dc:dc + 64],
                                                    pt[:, dc:dc + 64],
                                                    msk[:], ALU.mult)
                        if kb == 0:
                            nc.vector.tensor_copy(ptsum[:, 0:512], pt[:, 0:512])
                        else:
                            nc.vector.tensor_tensor(
                                ptsum[:, su:512], ptsum[:, su:512],
                                pt[:, h * 512 + su:(h + 1) * 512], ALU.add)
                        nc.tensor.matmul(
                            psy_u[:, su:512],
                            vbd4[m][:, kb // 2, kb % 2, :],
                            pt[:, h * 512 + su:(h + 1) * 512],
                            start=(kb == 0), stop=(kb == kbmax - 1),
                            skip_group_check=True)

                def tail(m=m, tstart=tstart, psy_u=psy_u, ptsum=ptsum):
                    psd = scrP.tile([128, 512], f32, tag="scr", name="psd")
                    nc.tensor.matmul(psd[0:2, :], ob1[:], ptsum[:, 0:512],
                                     start=True, stop=True)
                    rc = rcp.tile([2, 512], f32r, tag="rc", name="rct")
                    nc.vector.tensor_copy(rc[:], psd[0:2, :])
                    bc = scrP.tile([128, 512], f32, tag="scr", name="bct")
                    nc.tensor.matmul(bc[:], o2t[:], rc[:], start=True, stop=True)
                    bcs = rcp.tile([128, 512], f32, tag="bcs", name="bcst")
                    nc.vector.reciprocal_approx_fast(bcs[:], bc[:])
                    nc.vector.tensor_tensor(ysb[m][:, tstart:tstart + 512],
                                            psy_u[:], bcs[:], ALU.mult)
                pending.append(tail)

        def s4(jp):
            flush_pending()
            for tb in range(8):
                t0 = jp * 8 + tb
                ps = psaP.tile([128, 1024], f32, tag="psa", name=f"ps4_{t0}")
                for oc in range(2):
                    for k in range(4):
                        nc.tensor.matmul(
                            ps[:, oc * 512:(oc + 1) * 512],
                            ysb[k][:, t0 * 128:(t0 + 1) * 128],
                            wpt[k][:, oc * 512:(oc + 1) * 512],
                            start=(k == 0), stop=(k == 3))
                o_ = obp.tile([128, 1024], f32, tag="ob", name=f"ob{t0}")
                nc.vector.tensor_copy(o_[:], ps[:])
                nc.scalar.dma_start(out=outp[t0 * 128:(t0 + 1) * 128, :], in_=o_[:])

        # ---- master schedule ----
        xt0 = load_x(0)
        for m in range(NP):
            s1_qk(0, m, xt0)
        for tl in range(8):
            s1_v(0, tl, xt0)
        for m in range(NP):
            s1_vshuffle(0, m)
        xt1 = load_x(1)
        # jp0 attention interleaved with th1 projections (pure-PE filler)
        for m in range(NP):
            s2_chain(m, 0)
            s1_qk(1, m, xt1)
            s1_v(1, 2 * m, xt1)
            s1_v(1, 2 * m + 1, xt1)
        for m in range(NP):
            s1_vshuffle(1, m)
        s4(0)
        for m in range(NP):
            s2_chain(m, 1)
        s4(1)
        flush_pending()

    nc.compile()
    return nc


def _prep_core_inputs(x, w_qkv, w_proj, c):
    b, g = c // 2, c % 2
    bf = ml_dtypes.bfloat16
    scale = np.float32(D_HEAD ** -0.5)
    wq = w_qkv[512 * g:512 * (g + 1)] * scale
    wk = w_qkv[D_MODEL + 512 * g:D_MODEL + 512 * (g + 1)]
    wv = w_qkv[2 * D_MODEL + 512 * g:2 * D_MODEL + 512 * (g + 1)]
    r = np.arange(128) % 64
    j = np.arange(64)
    dmask = (r[:, None] <= j[None, :]).astype(np.float32)
    onesbd = np.zeros((128, 2), np.float32)
    onesbd[0:64, 0] = 1.0
    onesbd[64:128, 1] = 1.0
    ones2t = np.zeros((2, 128), np.float32)
    ones2t[0, 0:64] = 1.0
    ones2t[1, 64:128] = 1.0
    return {
        "xT": np.ascontiguousarray(x[b].T).astype(bf),
        "wqT": np.ascontiguousarray(wq.T).astype(bf),
        "wkT": np.ascontiguousarray(wk.T).astype(bf),
        "wvT": np.ascontiguousarray(wv.T).astype(bf),
        "wpT": np.ascontiguousarray(w_proj[:, 512 * g:512 * (g + 1)].T).astype(bf),
        "dmask": dmask.astype(bf),
        "onesbd": onesbd.astype(bf),
        "ones2t": ones2t,
    }


def kernel(x, w_qkv, w_proj):
    x = np.asarray(x)
    w_qkv = np.asarray(w_qkv)
    w_proj = np.asarray(w_proj)
    if "nc" not in _cache:
        _cache["nc"] = _build()
    nc = _cache["nc"]
    in_maps = [_prep_core_inputs(x, w_qkv, w_proj, c) for c in range(N_CORES)]
    res = run_bass_kernel_spmd(nc, in_maps, core_ids=list(range(N_CORES)))
    outs = [res.results[c]["out"] for c in range(N_CORES)]
    return np.stack([outs[2 * b] + outs[2 * b + 1] for b in range(B)], 0)


# revision 9
# speedup vs baseline: 1.5896x; 1.1253x over previous
import sys
import numpy as np

sys.path.insert(0, '/opt/trn_rl_repo')

import ml_dtypes
import concourse.bacc as bacc
import concourse.mybir as mybir
from concourse.bass_utils import run_bass_kernel_spmd
from concourse.tile import TileContext
from contextlib import ExitStack

f32 = mybir.dt.float32
f32r = mybir.dt.float32r
bf16 = mybir.dt.bfloat16
AF = mybir.ActivationFunctionType
ALU = mybir.AluOpType

D_MODEL = 1024
N_HEAD = 16
D_HEAD = 64
B = 4
T = 2048
N_CORES = 8
NP = 4            # head pairs per core
NKC = 8           # 128-row chunks over model dim (contraction)
NKB = 32          # key blocks of 64

_cache = {}


def _build():
    nc = bacc.Bacc()
    xT = nc.declare_dram_parameter("xT", [D_MODEL, T], bf16, isOutput=False)
    wall = nc.declare_dram_parameter("wall", [128, NKC * 1536], bf16, isOutput=False)
    wpall = nc.declare_dram_parameter("wpall", [128, 4 * D_MODEL], bf16, isOutput=False)
    dmask = nc.declare_dram_parameter("dmask", [128, 64], bf16, isOutput=False)
    onesbd = nc.declare_dram_parameter("onesbd", [128, 2], bf16, isOutput=False)
    ones2t = nc.declare_dram_parameter("ones2t", [2, 128], f32r, isOutput=False)
    outp = nc.declare_dram_parameter("out", [T, D_MODEL], f32, isOutput=True)

    with TileContext(nc) as tc, ExitStack() as X:
        pers = X.enter_context(tc.tile_pool(name="pers", bufs=1))
        xp = X.enter_context(tc.tile_pool(name="x", bufs=1))
        ptp = X.enter_context(tc.tile_pool(name="pt", bufs=4))
        psp = X.enter_context(tc.tile_pool(name="ptsum", bufs=2))
        rcp = X.enter_context(tc.tile_pool(name="rc", bufs=2))
        obp = X.enter_context(tc.tile_pool(name="ob", bufs=2))
        psaP = X.enter_context(tc.tile_pool(name="psa", bufs=2, space="PSUM"))
        psyP = X.enter_context(tc.tile_pool(name="psy", bufs=2, space="PSUM"))
        scrP = X.enter_context(tc.tile_pool(name="scr", bufs=2, space="PSUM"))

        qp = [pers.tile([128, T], bf16, name=f"qp{m}") for m in range(NP)]
        kbd = [pers.tile([128, NKB * 128], bf16, name=f"kbd{m}") for m in range(NP)]
        vbd = [pers.tile([128, NKB * 128], bf16, name=f"vbd{m}") for m in range(NP)]
        ysb = [pers.tile([128, T], bf16, name=f"ysb{m}") for m in range(NP)]
        vsb = pers.tile([128, 16 * 512], bf16, name="vsb")
        wallt = pers.tile([128, NKC * 1536], bf16, name="wallt")
        wall3 = wallt[:].rearrange("p (kc c) -> p kc c", kc=NKC)
        wqt = [wall3[:, k, 0:512] for k in range(NKC)]
        wkt = [wall3[:, k, 512:1024] for k in range(NKC)]
        wvt = [wall3[:, k, 1024:1536] for k in range(NKC)]
        wpallt = pers.tile([128, 4 * D_MODEL], bf16, name="wpallt")
        wp3 = wpallt[:].rearrange("p (k c) -> p k c", k=4)
        wpt = [wp3[:, k, :] for k in range(4)]
        msk = pers.tile([128, 64], bf16, name="msk")
        ob1 = pers.tile([128, 2], bf16, name="ob1")
        o2t = pers.tile([2, 128], f32r, name="o2t")

        # kbd block kb: [128, (2,64)] = block-diag(kA keys, kB keys) over d-partitions
        kbd3 = [kbd[m][:].rearrange("p (kb two s) -> p kb two s", two=2, s=64)
                for m in range(NP)]
        # vbd block kb=(tb,par): [128,128] = block-diag(vA, vB) over key-partitions
        vbd4 = [vbd[m][:].rearrange("p (tb par f) -> p tb par f", par=2, f=128)
                for m in range(NP)]
        vsb2 = vsb[:].rearrange("p (tb c) -> p tb c", tb=16)
        vsb3 = vsb[:].rearrange("p (tb m h d) -> p tb m h d", tb=16, m=4, h=2, d=64)

        # zero the off-diagonal quadrants once, before S1 evictions land
        for m in range(NP):
            nc.gpsimd.memset(kbd3[m][0:64, :, 1, :], 0.0)
            nc.gpsimd.memset(kbd3[m][64:128, :, 0, :], 0.0)
            nc.gpsimd.memset(vbd4[m][0:64, :, :, 64:128], 0.0)
            nc.gpsimd.memset(vbd4[m][64:128, :, :, 0:64], 0.0)

        nc.sync.dma_start(out=msk[:], in_=dmask[:, :])
        nc.sync.dma_start(out=ob1[:], in_=onesbd[:, :])
        nc.sync.dma_start(out=o2t[:], in_=ones2t[:, :])
        nc.scalar.dma_start(out=wallt[:], in_=wall[:, :])
        nc.scalar.dma_start(out=wpallt[:], in_=wpall[:, :])

        def load_x(th):
            xth = xp.tile([128, NKC * 1024], bf16, tag="x", name=f"x_{th}")
            x3 = xth[:].rearrange("p (kc t) -> p kc t", kc=NKC)
            nc.sync.dma_start(
                out=x3[:, :, :],
                in_=xT[:, th * 1024:(th + 1) * 1024].rearrange(
                    "(kc p) t -> p kc t", p=128))
            return [x3[:, k, :] for k in range(NKC)]

        def s1_qk(th, m, xt):
            hb = th * 1024
            for which in ("q", "k"):
                wt = wqt if which == "q" else wkt
                ps = psaP.tile([128, 1024], f32, tag="psa", name=f"ps{which}{m}_{th}")
                for kc in range(NKC):
                    for j in range(2):
                        nc.tensor.matmul(
                            ps[:, j * 512:(j + 1) * 512],
                            wt[kc][:, m * 128:(m + 1) * 128],
                            xt[kc][:, j * 512:(j + 1) * 512],
                            start=(kc == 0), stop=(kc == NKC - 1))
                if which == "q":
                    nc.scalar.activation(qp[m][:, hb:hb + 1024], ps[:], AF.Copy)
                else:
                    ps3 = ps[:].rearrange("p (kb s) -> p kb s", s=64)
                    nc.scalar.activation(
                        kbd3[m][0:64, th * 16:(th + 1) * 16, 0, :], ps3[0:64], AF.Copy)
                    nc.scalar.activation(
                        kbd3[m][64:128, th * 16:(th + 1) * 16, 1, :], ps3[64:128], AF.Copy)

        def s1_v(th, tl, xt):
            tb = th * 8 + tl
            ps = scrP.tile([128, 512], f32, tag="scr", name=f"psv{tb}")
            for kc in range(NKC):
                nc.tensor.matmul(ps[:], xt[kc][:, tl * 128:(tl + 1) * 128],
                                 wvt[kc], start=(kc == 0), stop=(kc == NKC - 1))
            nc.scalar.activation(vsb2[:, tb, :], ps[:], AF.Copy)

        def s1_vshuffle(th, m):
            tbs = slice(th * 8, (th + 1) * 8)
            nc.sync.dma_start(out=vbd4[m][0:64, tbs, 0, 0:64],
                              in_=vsb3[0:64, tbs, m, 0, :])
            nc.sync.dma_start(out=vbd4[m][64:128, tbs, 1, 64:128],
                              in_=vsb3[64:128, tbs, m, 1, :])
            nc.sync.dma_start(out=vbd4[m][64:128, tbs, 0, 64:128],
                              in_=vsb3[0:64, tbs, m, 1, :])
            nc.sync.dma_start(out=vbd4[m][0:64, tbs, 1, 0:64],
                              in_=vsb3[64:128, tbs, m, 0, :])

        pending = []

        def flush_pending():
            for f in pending:
                f()
            pending.clear()

        def s2_chain(m, jp):
            for u in range(2):
                flush_pending()
                tstart = jp * 1024 + u * 512
                kbmax = 16 * jp + 8 * u + 8
                psy_u = psyP.tile([128, 512], f32, tag="psy", name=f"psy{m}_{jp}_{u}")
                ptsum = psp.tile([128, 512], bf16, tag="ptsum", name=f"pts{m}_{jp}_{u}")
                for pi in range(0, kbmax, 2):
                    pair = [kb for kb in (pi, pi + 1) if kb < kbmax]
                    psa = psaP.tile([128, 1024], f32, tag="psa", name=f"psa{m}_{jp}_{u}_{pi}")
                    pt = ptp.tile([128, 1024], bf16, tag="pt", name=f"pt{m}_{jp}_{u}_{pi}")
                    sus = []
                    for h, kb in enumerate(pair):
                        su = max(0, 64 * kb - tstart)
                        sus.append(su)
                        nc.tensor.matmul(
                            psa[:, h * 512 + su:(h + 1) * 512],
                            kbd3[m][:, kb, :, :],
                            qp[m][:, tstart + su:tstart + 512],
                            start=True, stop=True)
                    if len(pair) == 2 and sus[0] == sus[1]:
                        p3i = psa[:].rearrange("p (g c) -> p g c", g=2)
                        p3o = pt[:].rearrange("p (g c) -> p g c", g=2)
                        nc.scalar.activation(p3o[:, :, sus[0]:512],
                                             p3i[:, :, sus[0]:512], AF.Exp)
                    else:
                        for h, kb in enumerate(pair):
                            su = sus[h]
                            nc.scalar.activation(
                                pt[:, h * 512 + su:(h + 1) * 512],
                                psa[:, h * 512 + su:(h + 1) * 512], AF.Exp)
                    for h, kb in enumerate(pair):
                        su = sus[h]
                        if 64 * kb >= tstart:
                            dc = h * 512 + 64 * kb - tstart
                            nc.gpsimd.tensor_tensor(pt[:, dc:dc + 64],
                                                    pt[:, dc:dc + 64],
                                                    msk[:], ALU.mult)
                        if kb == 0:
                            nc.vector.tensor_copy(ptsum[:, 0:512], pt[:, 0:512])
                        else:
                            nc.vector.tensor_tensor(
                                ptsum[:, su:512], ptsum[:, su:512],
                                pt[:, h * 512 + su:(h + 1) * 512], ALU.add)
                        nc.tensor.matmul(
                            psy_u[:, su:512],
                            vbd4[m][:, kb // 2, kb % 2, :],
                            pt[:, h * 512 + su:(h + 1) * 512],
                            start=(kb == 0), stop=(kb == kbmax - 1),
                            skip_group_check=True)

                def tail(m=m, tstart=tstart, psy_u=psy_u, ptsum=ptsum):
                    psd = scrP.tile([128, 512], f32, tag="scr", name="psd")
                    nc.tensor.matmul(psd[0:2, :], ob1[:], ptsum[:, 0:512],
                                     start=True, stop=True)
                    rc = rcp.tile([2, 512], f32r, tag="rc", name="rct")
                    nc.vector.tensor_copy(rc[:], psd[0:2, :])
                    bc = scrP.tile([128, 512], f32, tag="scr", name="bct")
                    nc.tensor.matmul(bc[:], o2t[:], rc[:], start=True, stop=True)
                    bcs = rcp.tile([128, 512], f32, tag="bcs", name="bcst")
                    nc.vector.reciprocal_approx_fast(bcs[:], bc[:])
                    nc.vector.tensor_tensor(ysb[m][:, tstart:tstart + 512],
                                            psy_u[:], bcs[:], ALU.mult)
                pending.append(tail)

        def s4(jp):
            flush_pending()
            for tb in range(8):
                t0 = jp * 8 + tb
                ps = psaP.tile([128, 1024], f32, tag="psa", name=f"ps4_{t0}")
                for oc in range(2):
                    for k in range(4):
                        nc.tensor.matmul(
                            ps[:, oc * 512:(oc + 1) * 512],
                            ysb[k][:, t0 * 128:(t0 + 1) * 128],
                            wpt[k][:, oc * 512:(oc + 1) * 512],
                            start=(k == 0), stop=(k == 3))
                o_ = obp.tile([128, 1024], f32, tag="ob", name=f"ob{t0}")
                nc.vector.tensor_copy(o_[:], ps[:])
                nc.gpsimd.dma_start(out=outp[t0 * 128:(t0 + 1) * 128, :], in_=o_[:])

        # ---- master schedule ----
        xt0 = load_x(0)
        for m in range(NP):
            s1_qk(0, m, xt0)
        for tl in range(8):
            s1_v(0, tl, xt0)
        for m in range(NP):
            s1_vshuffle(0, m)
        xt1 = load_x(1)
        # jp0 attention interleaved with th1 projections (pure-PE filler)
        for m in range(NP):
            s2_chain(m, 0)
            s1_qk(1, m, xt1)
            s1_v(1, 2 * m, xt1)
            s1_v(1, 2 * m + 1, xt1)
        for m in range(NP):
            s1_vshuffle(1, m)
        s4(0)
        for m in range(NP):
            s2_chain(m, 1)
        s4(1)
        flush_pending()

    nc.compile()
    return nc


def _prep_core_inputs(x, w_qkv, w_proj, c):
    b, g = c // 2, c % 2
    bf = ml_dtypes.bfloat16
    scale = np.float32(D_HEAD ** -0.5)
    wq = w_qkv[512 * g:512 * (g + 1)] * scale
    wk = w_qkv[D_MODEL + 512 * g:D_MODEL + 512 * (g + 1)]
    wv = w_qkv[2 * D_MODEL + 512 * g:2 * D_MODEL + 512 * (g + 1)]
    r = np.arange(128) % 64
    j = np.arange(64)
    dmask = (r[:, None] <= j[None, :]).astype(np.float32)
    onesbd = np.zeros((128, 2), np.float32)
    onesbd[0:64, 0] = 1.0
    onesbd[64:128, 1] = 1.0
    ones2t = np.zeros((2, 128), np.float32)
    ones2t[0, 0:64] = 1.0
    ones2t[1, 64:128] = 1.0
    wqT = wq.T.reshape(NKC, 128, 512)
    wkT = wk.T.reshape(NKC, 128, 512)
    wvT = wv.T.reshape(NKC, 128, 512)
    wall = np.concatenate([wqT, wkT, wvT], axis=2).transpose(1, 0, 2).reshape(128, -1)
    wpT = w_proj[:, 512 * g:512 * (g + 1)].T.reshape(4, 128, D_MODEL)
    wpall = wpT.transpose(1, 0, 2).reshape(128, -1)
    return {
        "xT": np.ascontiguousarray(x[b].T).astype(bf),
        "wall": np.ascontiguousarray(wall).astype(bf),
        "wpall": np.ascontiguousarray(wpall).astype(bf),
        "dmask": dmask.astype(bf),
        "onesbd": onesbd.astype(bf),
        "ones2t": ones2t,
    }


def kernel(x, w_qkv, w_proj):
    x = np.asarray(x)
    w_qkv = np.asarray(w_qkv)
    w_proj = np.asarray(w_proj)
    if "nc" not in _cache:
        _cache["nc"] = _build()
    nc = _cache["nc"]
    in_maps = [_prep_core_inputs(x, w_qkv, w_proj, c) for c in range(N_CORES)]
    res = run_bass_kernel_spmd(nc, in_maps, core_ids=list(range(N_CORES)))
    outs = [res.results[c]["out"] for c in range(N_CORES)]
    return np.stack([outs[2 * b] + outs[2 * b + 1] for b in range(B)], 0)
